# revision 1
# baseline (speedup 1.0000x reference)
"""Trainium2 Bass kernel for nn_LinearDiffusion (truncated Taylor expm(a) @ x).

Math: a = row-normalized symmetric scatter of per-head edge weights onto an
(H, N, N) zero tensor; result = sum_{i=0..6} a^i x / i! with x = h reshaped
per-head.

Strategy (8 NeuronCores, one chip):
  * The adjacency is ~0.4% dense; the dense einsum would stream 1 GB of
    matrix 6x. Instead: sparse formulation with the pattern preprocessed on
    host into per-core tables.
  * Node features of all 4 heads are kept together: one node row = 64 fp32
    = 256 B, the exact granularity of `dma_gather`.
  * Shard by destination row: core k owns rows [k*1024, (k+1)*1024).
    Edge entries (r, c, w) sorted by r, padded into 128-edge chunks that
    each scatter into one 128-row block.
  * Per iteration, per core:
      1. dma_gather of x[src] rows (256 B each) from a DRAM copy of x
      2. VectorE: weighted product, split hi/lo fp16 (exact to ~2^-22)
      3. TensorE: per chunk, one-hot scatter matrix (fp8, SBUF-resident)
         x [hi|lo] rhs -> accumulate the block's (128, 128) PSUM tile
      4. evacuate PSUM, accumulate Taylor term, AllGather new x
  * Only the table *data* differs per core, so one SPMD program serves all
    8 cores; per-core tables arrive as inputs.
"""

import math
from dataclasses import dataclass

import numpy as np

import concourse.bass as bass  # noqa: F401  (kept for callers)
import concourse.tile as tile
from concourse import bacc, mybir
from concourse.bass_utils import run_bass_kernel_spmd

# ----------------------------------------------------------------- config

N, H, E, D = 8192, 4, 131072, 64
d = D // H
NCORES = 8
BLK = 128  # dst-block size == PE stationary width
K_TAYLOR = 6


@dataclass(frozen=True)
class Cfg:
    n: int = N
    n_cores: int = NCORES
    hi_lo_split: bool = True  # False -> single fp16 product (faster, ~5e-4 err)

    @property
    def rows_per_core(self):
        return self.n // self.n_cores

    @property
    def blocks_per_core(self):
        return self.rows_per_core // BLK


# ----------------------------------------------------------- preprocessing


def _entries(e, src, dst, n):
    """Unique symmetric entries with 'last write wins' duplicate semantics,
    matching jax's .at[].set() on CPU. Returns (rows, cols, w[H, nnz])."""
    src = src.astype(np.int64)
    dst = dst.astype(np.int64)
    n_edges = len(src)
    keys = np.concatenate([src * n + dst, dst * n + src])
    eid = np.concatenate([np.arange(n_edges), np.arange(n_edges)])
    order = np.arange(2 * n_edges)
    perm = np.lexsort((-order, keys))
    k_sorted = keys[perm]
    first = np.ones(len(k_sorted), dtype=bool)
    first[1:] = k_sorted[1:] != k_sorted[:-1]
    win = perm[first]
    ukeys = k_sorted[first]
    rows = (ukeys // n).astype(np.int64)
    cols = (ukeys % n).astype(np.int64)
    weids = eid[win]
    vals = e[:, weids].astype(np.float64)  # (H, nnz)
    nheads = e.shape[0]
    rowsum = np.zeros((nheads, n), dtype=np.float64)
    for hh in range(nheads):
        rowsum[hh] = np.bincount(rows, weights=vals[hh], minlength=n)
    w = (vals / rowsum[:, rows]).astype(np.float32)
    return rows, cols, w


def _make_tables(e, src, dst, cfg: Cfg):
    """Per-core device tables. Returns (tables, nch) where tables is a list
    over cores of dicts with keys idx (int16), w4 (fp32), sca (fp8)."""
    import ml_dtypes

    n = cfg.n
    rows, cols, w = _entries(e, src, dst, n)
    nheads = w.shape[0]
    bpc = cfg.blocks_per_core

    order = np.argsort(rows, kind="stable")
    rows_s, cols_s, w_s = rows[order], cols[order], w[:, order]
    blk = rows_s // BLK
    nblocks = n // BLK
    starts = np.searchsorted(blk, np.arange(nblocks + 1))
    bcnt = np.diff(starts)
    bmax = int(np.ceil(bcnt.max() / 128))  # chunks per block, uniform
    nch = bpc * bmax

    tables = []
    for k in range(cfg.n_cores):
        idx = np.zeros((nch, 128), dtype=np.int16)
        w4 = np.zeros((128, nch, nheads), dtype=np.float32)
        sca = np.zeros((128, nch, 128), dtype=ml_dtypes.float8_e4m3fn)
        for j in range(bpc):
            b = k * bpc + j
            s, cnt = starts[b], bcnt[b]
            sl = slice(s, s + cnt)
            eloc = np.arange(cnt)
            c_local = j * bmax + eloc // 128
            p = eloc % 128
            idx[c_local, p] = cols_s[sl].astype(np.int16)
            w4[p, c_local, :] = w_s[:, sl].T
            m = rows_s[sl] - b * BLK
            sca[p, c_local, m] = 1.0
        # dma_gather index layout: logical index i -> [i % 16, i // 16],
        # replicated across the 8 groups of 16 partitions.
        seq = idx.reshape(-1)  # logical order: i = c*128 + p
        wrapped = seq.reshape(-1, 16).T  # (16, nch*8)
        idx_t = np.tile(wrapped, (8, 1))  # (128, nch*8)
        tables.append(
            {
                "idx": np.ascontiguousarray(idx_t),
                "w4": np.ascontiguousarray(w4.reshape(128, nch * nheads)),
                "sca": np.ascontiguousarray(sca.reshape(128, nch * 128)),
            }
        )
    return tables, nch


# ------------------------------------------------------------ bass program

_FP32 = mybir.dt.float32
_FP16 = mybir.dt.float16
_FP8 = mybir.dt.float8e4
_I16 = mybir.dt.int16


def _build_program(cfg: Cfg, nch: int):
    n = cfg.n
    bpc = cfg.blocks_per_core
    bmax = nch // bpc
    rpc = cfg.rows_per_core
    nc = bacc.Bacc(
        "TRN2",
        target_bir_lowering=False,
        debug=False,
        num_devices=cfg.n_cores,
    )

    xin = nc.dram_tensor("xin", [n, D], _FP32, kind="ExternalInput").ap()
    x0s_d = nc.dram_tensor("x0s", [rpc, D], _FP32, kind="ExternalInput").ap()
    idx_d = nc.dram_tensor("idx", [128, nch * 8], _I16, kind="ExternalInput").ap()
    w4_d = nc.dram_tensor("w4", [128, nch * H], _FP32, kind="ExternalInput").ap()
    sca_d = nc.dram_tensor("sca", [128, nch * 128], _FP8, kind="ExternalInput").ap()
    out_d = nc.dram_tensor("out", [rpc, D], _FP32, kind="ExternalOutput").ap()

    xall = nc.dram_tensor("xall", [n, D], _FP32, addr_space="Shared").ap()
    slice_in = nc.dram_tensor("slice_in", [rpc, D], _FP32).ap()

    groups = [list(range(cfg.n_cores))]

    # Sub-batch the per-iteration work so each dma_gather stays under the
    # SWDGE descriptor-ring capacity (~9k indices per call observed safe).
    halves = 1
    while nch // halves * 128 > 9216 or bpc % halves:
        halves += 1
        assert halves <= bpc, "cannot find sub-batch split"
    hbpc = bpc // halves  # blocks per sub-batch
    hch = nch // halves  # chunks per sub-batch

    with tile.TileContext(nc) as tc:
        with (
            tc.tile_pool(name="tables", bufs=1) as tp,
            tc.tile_pool(name="xg", bufs=2) as xgp,
            tc.tile_pool(name="xgw", bufs=2) as xgwp,
            tc.tile_pool(name="acc", bufs=1) as accp,
            tc.tile_pool(name="stage", bufs=2) as stp,
            tc.tile_pool(name="psum", bufs=4, space="PSUM") as pp,
        ):
            idx_sb = tp.tile([128, nch * 8], _I16)
            w4_sb = tp.tile([128, nch, H], _FP32)
            sca_sb = tp.tile([128, nch * 128], _FP8)
            nc.sync.dma_start(out=idx_sb[:], in_=idx_d)
            nc.sync.dma_start(
                out=w4_sb[:].rearrange("p c h -> p (c h)"), in_=w4_d
            )
            nc.sync.dma_start(out=sca_sb[:], in_=sca_d)

            # x0: full copy into the gather buffer + this core's slice into
            # the running Taylor accumulator (identity term).
            nc.sync.dma_start(out=xall, in_=xin)
            result = accp.tile([128, bpc, D], _FP32)
            nc.sync.dma_start(
                out=result[:],
                in_=x0s_d.rearrange("(j p) f -> p j f", p=128),
            )

            for it in range(1, K_TAYLOR + 1):
                coef = 1.0 / math.factorial(it)
                xnext = stp.tile([128, bpc, D], _FP32, tag="xnext")
                for hf in range(halves):
                    c0 = hf * hch
                    xg = xgp.tile([128, hch, D], _FP32, tag="xg")
                    nc.gpsimd.dma_gather(
                        xg[:],
                        xall,
                        idx_sb[:, c0 * 8 : (c0 + hch) * 8],
                        hch * 128,
                        hch * 128,
                        D,
                        single_packet=False,
                    )
                    # prod = xg * w4 (broadcast each head weight over d)
                    xg4 = xg[:].rearrange("p c (h f) -> p c h f", h=H)
                    w4v = (
                        w4_sb[:, c0 : c0 + hch, :]
                        .unsqueeze(3)
                        .to_broadcast([128, hch, H, d])
                    )
                    xgw = xgwp.tile([128, hch, 2 * D], _FP16, tag="xgw")
                    hi = xgw[:, :, 0:D].rearrange("p c (h f) -> p c h f", h=H)
                    lo = xgw[:, :, D : 2 * D].rearrange(
                        "p c (h f) -> p c h f", h=H
                    )
                    if cfg.hi_lo_split:
                        nc.vector.tensor_mul(xg4, xg4, w4v)
                        nc.scalar.copy(hi, xg4)
                        nc.vector.tensor_sub(lo, xg4, hi)
                    else:
                        nc.vector.tensor_mul(hi, xg4, w4v)
                        nc.vector.memset(xgw[:, :, D : 2 * D], 0.0)

                    for jj in range(hf * hbpc, (hf + 1) * hbpc):
                        ps = pp.tile([128, 2 * D], _FP32, tag="ps")
                        for b in range(bmax):
                            c = jj * bmax + b
                            nc.tensor.matmul(
                                ps[:],
                                lhsT=sca_sb[:, c * 128 : (c + 1) * 128],
                                rhs=xgw[:, c - c0, :],
                                start=(b == 0),
                                stop=(b == bmax - 1),
                            )
                        nc.scalar.copy(xnext[:, jj, :], ps[:, 0:D])
                        nc.vector.tensor_add(
                            xnext[:, jj, :], xnext[:, jj, :], ps[:, D : 2 * D]
                        )
                        nc.vector.scalar_tensor_tensor(
                            result[:, jj, :],
                            xnext[:, jj, :],
                            coef,
                            result[:, jj, :],
                            op0=mybir.AluOpType.mult,
                            op1=mybir.AluOpType.add,
                        )
                if it < K_TAYLOR:
                    nc.sync.dma_start(
                        out=slice_in.rearrange("(j p) f -> p j f", p=128),
                        in_=xnext[:],
                    )
                    nc.gpsimd.collective_compute(
                        "AllGather",
                        mybir.AluOpType.bypass,
                        replica_groups=groups,
                        ins=[slice_in],
                        outs=[xall],
                    )

            nc.sync.dma_start(
                out=out_d.rearrange("(j p) f -> p j f", p=128),
                in_=result[:],
            )

    nc.compile()
    return nc


# ------------------------------------------------------------------ driver

_CACHE = {}


def _get_program(cfg: Cfg, nch: int):
    key = (cfg, nch)
    if key not in _CACHE:
        _CACHE[key] = _build_program(cfg, nch)
    return _CACHE[key]


def _in_maps(x0, tables, cfg: Cfg):
    rpc = cfg.rows_per_core
    return [
        {
            "xin": x0,
            "x0s": np.ascontiguousarray(x0[k * rpc : (k + 1) * rpc]),
            "idx": t["idx"],
            "w4": t["w4"],
            "sca": t["sca"],
        }
        for k, t in enumerate(tables)
    ]


def run(h, e, src, dst, cfg: Cfg = Cfg(), trace: bool = False):
    """Full pipeline: preprocess, build/compile (cached), execute, assemble."""
    h = np.asarray(h, dtype=np.float32)
    e = np.asarray(e, dtype=np.float32)
    src = np.asarray(src)
    dst = np.asarray(dst)
    nheads = e.shape[0]
    n = h.shape[0]
    dd = h.shape[1] // nheads
    assert (n, nheads, dd) == (cfg.n, H, d), (n, nheads, dd)

    tables, nch = _make_tables(e, src, dst, cfg)
    x0 = np.ascontiguousarray(
        h.reshape(nheads, n, dd).transpose(1, 0, 2).reshape(n, nheads * dd)
    )
    nc = _get_program(cfg, nch)
    res = run_bass_kernel_spmd(
        nc,
        _in_maps(x0, tables, cfg),
        list(range(cfg.n_cores)),
        trace=trace,
    )
    out = np.concatenate(
        [res.results[k]["out"] for k in range(cfg.n_cores)], axis=0
    )
    # back to reference layout: (n, H, d) node-major -> (H, n, d) -> (N, D)
    out = np.ascontiguousarray(out.reshape(n, nheads, dd).transpose(1, 0, 2)).reshape(
        n, nheads * dd
    )
    return out, res


def kernel(h, e, src, dst):
    out, _ = run(h, e, src, dst)
    return out



# revision 8
# speedup vs baseline: 3.5652x; 3.5652x over previous
"""Trainium2 Bass kernel for nn_LinearDiffusion (truncated Taylor expm(a) @ x).

Math: a = row-normalized symmetric scatter of per-head edge weights onto an
(H, N, N) zero tensor; result = sum_{i=0..6} a^i x / i! with x = h reshaped
per-head.

Strategy (8 NeuronCores, one chip) — v2, TensorE-gather:
  * x (8192 x 64 fp16, all heads together) lives in SBUF on every core; the
    per-edge gather x[src] is computed by TensorE one-hot matmuls from the
    SBUF-resident copy instead of per-edge DMA (the v1 bottleneck: GpSimd
    SWDGE descriptor generation at ~8 ns/edge, 1.7 ms total).
  * Core k owns dst rows [k*1024, (k+1)*1024) = 8 blocks of 128.  Edges are
    binned per (dst block jb, src block sb) cell; each cell gets a fixed
    64-slot capacity (mean occupancy ~64).  A 128-slot chunk = 2 cells.
      - gather:  per cell, one matmul  psum[c0:c0+64, chunk] +=
                 gmat[:, cell]^T @ xsb[:, sb, :]   (gmat: fp8 one-hot of
                 src_local, zero-padded; writes every PSUM byte -> no junk)
      - weights: one DVE multiply per 8-chunk PSUM bank with the per-head
                 w4 table (broadcast over the 16 feats of each head), fp16 out
      - scatter: per chunk, one matmul into the iteration's output bank
                 pout[:, jb*64:+64] += sca[:, chunk]^T @ xgw  (sca: fp8
                 one-hot of dst_local; PSUM accumulation across chunks)
  * Cell overflow (~5% of edges) spills to the old dma_gather path (runs on
    the otherwise-idle GpSimd, overlapped with TensorE work), gathering
    256 B token pairs from the fp16 exchange buffer; a zeroed half in the
    w4 spill table selects the wanted node of each pair, two scatter
    matmuls per spill chunk (one per parity half) accumulate into pout.
  * Between iterations: AllGather of the fp16 x slices (128 KB/rank) and a
    single strided DMA reload of xsb.  Output accumulates in fp32.
  * One SPMD program serves all 8 cores: the (jb, sb) cell grid is shared;
    only table data differs per core.
"""

import math
from dataclasses import dataclass

import numpy as np

import concourse.bass as bass  # noqa: F401  (kept for callers)
import concourse.tile as tile
from concourse import bacc, mybir
from concourse.bass_utils import run_bass_kernel_spmd

# ----------------------------------------------------------------- config

N, H, E, D = 8192, 4, 131072, 64
d = D // H
NCORES = 8
BLK = 128          # dst-block size == PSUM partition width
NSB = N // BLK     # src blocks (64)
CELL = 64          # slots per (jb, sb) cell
JBS = N // NCORES // BLK   # dst blocks per core (8)
MAIN_CH = NSB * CELL // 128  # main chunks per dst block (32)
K_TAYLOR = 6


@dataclass(frozen=True)
class Cfg:
    n: int = N
    n_cores: int = NCORES
    kt: int = K_TAYLOR  # Taylor depth (debug)


# ----------------------------------------------------------- preprocessing


def _entries(e, src, dst, n):
    """Unique symmetric entries with 'last write wins' duplicate semantics,
    matching jax's .at[].set() on CPU. Returns (rows, cols, w[H, nnz])."""
    src = src.astype(np.int64)
    dst = dst.astype(np.int64)
    n_edges = len(src)
    keys = np.concatenate([src * n + dst, dst * n + src])
    eid = np.concatenate([np.arange(n_edges), np.arange(n_edges)])
    order = np.arange(2 * n_edges)
    perm = np.lexsort((-order, keys))
    k_sorted = keys[perm]
    first = np.ones(len(k_sorted), dtype=bool)
    first[1:] = k_sorted[1:] != k_sorted[:-1]
    win = perm[first]
    ukeys = k_sorted[first]
    rows = (ukeys // n).astype(np.int64)
    cols = (ukeys % n).astype(np.int64)
    weids = eid[win]
    vals = e[:, weids].astype(np.float64)  # (H, nnz)
    nheads = e.shape[0]
    rowsum = np.zeros((nheads, n), dtype=np.float64)
    for hh in range(nheads):
        rowsum[hh] = np.bincount(rows, weights=vals[hh], minlength=n)
    w = (vals / rowsum[:, rows]).astype(np.float32)
    return rows, cols, w


def _make_tables(e, src, dst, cfg: Cfg):
    """Per-core device tables. Returns (tables, S_sp)."""
    import ml_dtypes

    n = cfg.n
    rows, cols, w = _entries(e, src, dst, n)
    rpc = n // cfg.n_cores

    # Pass 1: spill sizes -> shared spill chunk count S_sp.
    spill_max = 0
    percore = []
    for k in range(cfg.n_cores):
        m = (rows >= k * rpc) & (rows < (k + 1) * rpc)
        r = rows[m] - k * rpc
        c = cols[m]
        wv = w[:, m]
        jb = r // BLK
        dl = r % BLK
        sb = c // BLK
        sl = c % BLK
        # stable order by (jb, sb)
        o = np.lexsort((sb, jb))
        jb, dl, sb, sl, wv = jb[o], dl[o], sb[o], sl[o], wv[:, o]
        # rank within cell
        cell = jb * NSB + sb
        # positions within each cell (cells are contiguous after the sort)
        idx_in_cell = np.arange(len(cell)) - np.searchsorted(cell, cell)
        main = idx_in_cell < CELL
        percore.append((jb, dl, sb, sl, wv, idx_in_cell, main))
        for j in range(JBS):
            spill_max = max(spill_max, int(np.sum(~main & (jb == j))))
    S_sp = max(1, -(-spill_max // 128))  # spill chunks per dst block
    nch = MAIN_CH + S_sp                 # chunks per dst block
    ntok = JBS * S_sp * 128              # spill tokens per core

    tables = []
    for k in range(cfg.n_cores):
        jb, dl, sb, sl, wv, ic, main = percore[k]
        gmat = np.zeros((128, JBS * NSB * CELL), dtype=ml_dtypes.float8_e4m3fn)
        sca = np.zeros((128, JBS * nch * 128), dtype=ml_dtypes.float8_e4m3fn)
        w4m = np.zeros((128, JBS * MAIN_CH, H), dtype=np.float32)
        w4sp = np.zeros((128, JBS * S_sp, 2, H), dtype=np.float32)
        tok = np.zeros(ntok, dtype=np.int64)

        # main edges
        jm, dm, sbm, slm, wm, im = (
            jb[main], dl[main], sb[main], sl[main], wv[:, main], ic[main])
        # slot position within the chunk: cell parity picks the 64-half
        p = (sbm % 2) * CELL + im
        chunk_g = jm * nch + sbm // 2          # global chunk id
        gcol = jm * (NSB * CELL) + sbm * CELL + im
        gmat[slm, gcol] = 1.0
        sca[p, chunk_g * 128 + dm] = 1.0
        w4m[p, jm * MAIN_CH + sbm // 2, :] = wm.T

        # spill edges
        js, ds, ws = jb[~main], dl[~main], wv[:, ~main]
        srcs = sb[~main] * BLK + sl[~main]     # global src node
        for j in range(JBS):
            mj = js == j
            cnt = int(mj.sum())
            pos = np.arange(cnt)
            sc = pos // 128                    # spill chunk within jb
            pp = pos % 128
            src_j = srcs[mj]
            kk = src_j // rpc
            within = src_j % rpc
            jbs_ = within // BLK
            psrc = within % BLK
            token = (kk * 128 + psrc) * (JBS // 2) + jbs_ // 2
            par = jbs_ % 2
            tok[(j * S_sp + sc) * 128 + pp] = token
            w4sp[pp, j * S_sp + sc, par, :] = ws[:, mj].T
            sca[pp, (j * nch + MAIN_CH + sc) * 128 + ds[mj]] = 1.0

        # dma_gather index layout: wrapped in 16 partitions, replicated x8
        wrapped = tok.reshape(-1, 16).T.astype(np.int16)
        idx_t = np.tile(wrapped, (8, 1))
        tables.append(
            {
                "gmat": np.ascontiguousarray(gmat),
                "sca": np.ascontiguousarray(sca),
                "w4m": np.ascontiguousarray(w4m.reshape(128, -1)),
                "w4sp": np.ascontiguousarray(w4sp.reshape(128, -1)),
                "idx": np.ascontiguousarray(idx_t),
            }
        )
    return tables, S_sp


def _emulate(tables, S_sp, xe, cfg: Cfg):
    """Numpy emulation of one iteration y = A @ x using the device tables.
    xe: [1024, 512] fp16 exchange-layout x.  Returns y [8192, 64] float32."""
    nch = MAIN_CH + S_sp
    xflat = np.ascontiguousarray(xe).reshape(-1, 128)  # token rows
    out = np.zeros((NCORES, 128, JBS, 64), dtype=np.float32)
    # xsb[p, b, f] = x[b*128+p, f] ; from xe rows (k*128+p, jb*64+f)
    xsb = (
        xe.reshape(NCORES, 128, JBS, 64).transpose(1, 0, 2, 3)
        .reshape(128, NSB, 64).astype(np.float32)
    )
    for k in range(cfg.n_cores):
        t = tables[k]
        gmat = t["gmat"].astype(np.float32)
        sca = t["sca"].astype(np.float32)
        w4m = t["w4m"].reshape(128, JBS * MAIN_CH, H)
        w4sp = t["w4sp"].reshape(128, JBS * S_sp, 2, H)
        # spill gather
        seq = t["idx"][:16].T.reshape(-1)  # un-wrap
        xg_sp = xflat[seq].reshape(JBS * S_sp, 128, 128).transpose(1, 0, 2)
        for j in range(JBS):
            acc = np.zeros((128, 64), dtype=np.float32)
            for c in range(MAIN_CH):
                pg = np.zeros((128, 64), dtype=np.float32)
                for half in range(2):
                    sb = c * 2 + half
                    g = gmat[:, j * NSB * CELL + sb * CELL:][:, :CELL]
                    pg[half * 64:(half + 1) * 64] = g.T @ xsb[:, sb, :]
                w4 = w4m[:, j * MAIN_CH + c, :]  # [128, H]
                xgw = (pg.reshape(128, H, d)
                       * w4[:, :, None]).reshape(128, 64).astype(np.float16)
                s = sca[:, (j * nch + c) * 128:][:, :128]
                acc += s.T @ xgw.astype(np.float32)
            for sc in range(S_sp):
                xg = xg_sp[:, j * S_sp + sc, :].astype(np.float32)
                w4 = w4sp[:, j * S_sp + sc, :, :]  # [128, 2, H]
                xgw = (xg.reshape(128, 2, H, d)
                       * w4[:, :, :, None]).reshape(128, 128)
                xgw = xgw.astype(np.float16).astype(np.float32)
                s = sca[:, (j * nch + MAIN_CH + sc) * 128:][:, :128]
                acc += s.T @ (xgw[:, :64] + xgw[:, 64:])
            out[k, :, j, :] = acc
    # out[k, p, jb, f] -> node (k*1024 + jb*128 + p)
    return out.transpose(0, 2, 1, 3).reshape(N, 64)


# ------------------------------------------------------------ bass program

_FP32 = mybir.dt.float32
_FP16 = mybir.dt.float16
_FP8 = mybir.dt.float8e4
_I16 = mybir.dt.int16


def _build_program(cfg: Cfg, S_sp: int):
    kt = cfg.kt
    nch = MAIN_CH + S_sp
    ntok = JBS * S_sp * 128
    nc = bacc.Bacc(
        "TRN2",
        target_bir_lowering=False,
        debug=False,
        num_devices=cfg.n_cores,
    )

    xe0_d = nc.dram_tensor("xe0", [1024, 512], _FP16, kind="ExternalInput").ap()
    x0s_d = nc.dram_tensor("x0s", [128, 512], _FP32, kind="ExternalInput").ap()
    gmat_d = nc.dram_tensor(
        "gmat", [128, JBS * NSB * CELL], _FP8, kind="ExternalInput").ap()
    sca_d = nc.dram_tensor(
        "sca", [128, JBS * nch * 128], _FP8, kind="ExternalInput").ap()
    w4m_d = nc.dram_tensor(
        "w4m", [128, JBS * MAIN_CH * H], _FP32, kind="ExternalInput").ap()
    w4sp_d = nc.dram_tensor(
        "w4sp", [128, JBS * S_sp * 2 * H], _FP32, kind="ExternalInput").ap()
    idx_d = nc.dram_tensor(
        "idx", [128, ntok // 16], _I16, kind="ExternalInput").ap()
    out_d = nc.dram_tensor("out", [128, 512], _FP32, kind="ExternalOutput").ap()

    slice_in = nc.dram_tensor("slice_in", [128, 512], _FP16).ap()
    xallE = nc.dram_tensor(
        "xallE", [1024, 512], _FP16, addr_space="Shared").ap()

    groups = [list(range(cfg.n_cores))]

    with tile.TileContext(nc) as tc:
        with (
            tc.tile_pool(name="tables", bufs=1) as tp,
            tc.tile_pool(name="xgw", bufs=3) as xgwp,
            tc.tile_pool(name="psg", bufs=3, space="PSUM") as pgp,
            tc.tile_pool(name="pso", bufs=2, space="PSUM") as pop,
        ):
            gmat_sb = [tp.tile([128, NSB * CELL], _FP8, tag=f"gm{j}",
                               name=f"gmat_sb{j}")
                       for j in range(JBS)]
            sca_sb = [tp.tile([128, nch * 128], _FP8, tag=f"sc{j}",
                              name=f"sca_sb{j}")
                      for j in range(JBS)]
            w4m_sb = tp.tile([128, JBS * MAIN_CH, H], _FP32)
            w4sp_sb = tp.tile([128, JBS * S_sp, 2 * H], _FP32)
            idx_sb = tp.tile([128, ntok // 16], _I16)
            xsb = tp.tile([128, NSB, 64], _FP16)
            xnext = tp.tile([128, JBS * 64], _FP16)
            result = tp.tile([128, JBS * 64], _FP32)
            xg_sp = tp.tile([128, JBS * S_sp, 128], _FP16)
            xgw_sp = tp.tile([128, JBS * S_sp, 128], _FP16)
            z128 = tp.tile([128, 128], _FP8)
            nc.vector.memset(z128[:], 0.0)

            for j in range(JBS):
                nc.sync.dma_start(
                    out=gmat_sb[j][:],
                    in_=gmat_d[:, j * NSB * CELL:(j + 1) * NSB * CELL])
                nc.sync.dma_start(
                    out=sca_sb[j][:],
                    in_=sca_d[:, j * nch * 128:(j + 1) * nch * 128])
            nc.sync.dma_start(
                out=w4m_sb[:].rearrange("p c h -> p (c h)"), in_=w4m_d)
            nc.sync.dma_start(
                out=w4sp_sb[:].rearrange("p c h -> p (c h)"), in_=w4sp_d)
            nc.sync.dma_start(out=idx_sb[:], in_=idx_d)
            # x0 into xsb ([p, b, f] <- xe0 rows (k*128+p, jb*64+f))
            nc.sync.dma_start(
                out=xsb[:].rearrange("p (k j) f -> p k (j f)", k=NCORES),
                in_=xe0_d.rearrange("(k p) f -> p k f", p=128),
            )
            nc.sync.dma_start(out=result[:], in_=x0s_d)

            for it in range(1, kt + 1):
                coef = 1.0 / math.factorial(it)
                xsrc = xe0_d if it == 1 else xallE
                # ---- spill gather (GpSimd) + weight multiply
                nc.gpsimd.dma_gather(
                    xg_sp[:],
                    xsrc.rearrange("r (q e) -> (r q) e", e=128),
                    idx_sb[:],
                    ntok,
                    ntok,
                    128,
                    single_packet=False,
                )
                for par in range(2):
                    xg4 = xg_sp[:, :, par * 64:(par + 1) * 64].rearrange(
                        "p c (h f) -> p c h f", h=H)
                    wv = (
                        w4sp_sb[:]
                        .rearrange("p c (t h) -> p c t h", t=2)[:, :, par, :]
                        .unsqueeze(3)
                        .to_broadcast([128, JBS * S_sp, H, d])
                    )
                    og = xgw_sp[:, :, par * 64:(par + 1) * 64].rearrange(
                        "p c (h f) -> p c h f", h=H)
                    nc.vector.tensor_mul(og, xg4, wv)

                pout = pop.tile([128, JBS * 64], _FP32, tag="pout")
                # open the accumulation bank: zero matmul writes every byte
                # (hw clears has_written per-element, not per-bank)
                nc.tensor.matmul(
                    pout[:],
                    lhsT=z128[:],
                    rhs=xsb[:, 0:8, :],
                    start=True,
                    stop=False,
                )
                # ---- main cells
                for j in range(JBS):
                    for g in range(MAIN_CH // 8):
                        pg = pgp.tile([128, 8 * 64], _FP32, tag="pg")
                        for ci in range(8):
                            c = g * 8 + ci
                            for half in range(2):
                                sb = c * 2 + half
                                nc.tensor.matmul(
                                    pg[half * 64:(half + 1) * 64,
                                       ci * 64:(ci + 1) * 64],
                                    lhsT=gmat_sb[j][
                                        :, sb * CELL:(sb + 1) * CELL],
                                    rhs=xsb[:, sb, :],
                                    start=True,
                                    stop=True,
                                )
                        xgw = xgwp.tile([128, 8, 64], _FP16, tag="xgw")
                        pg4 = pg[:].rearrange("p (c h f) -> p c h f", c=8, h=H)
                        wv = (
                            w4m_sb[:, j * MAIN_CH + g * 8:
                                   j * MAIN_CH + (g + 1) * 8, :]
                            .unsqueeze(3)
                            .to_broadcast([128, 8, H, d])
                        )
                        nc.vector.tensor_mul(
                            xgw[:].rearrange("p c (h f) -> p c h f", h=H),
                            pg4, wv)
                        for ci in range(8):
                            c = g * 8 + ci
                            nc.tensor.matmul(
                                pout[:, j * 64:(j + 1) * 64],
                                lhsT=sca_sb[j][:, c * 128:(c + 1) * 128],
                                rhs=xgw[:, ci, :],
                                start=False,
                                stop=False,
                            )
                # ---- spill scatter (end of iteration; rhs ready early)
                for j in range(JBS):
                    for sc in range(S_sp):
                        for par in range(2):
                            last = (j == JBS - 1 and sc == S_sp - 1
                                    and par == 1)
                            nc.tensor.matmul(
                                pout[:, j * 64:(j + 1) * 64],
                                lhsT=sca_sb[j][
                                    :, (MAIN_CH + sc) * 128:
                                    (MAIN_CH + sc + 1) * 128],
                                rhs=xgw_sp[:, j * S_sp + sc,
                                           par * 64:(par + 1) * 64],
                                start=False,
                                stop=last,
                            )
                # ---- evacuate + Taylor accumulate
                nc.scalar.copy(xnext[:], pout[:])
                nc.vector.scalar_tensor_tensor(
                    result[:],
                    pout[:],
                    coef,
                    result[:],
                    op0=mybir.AluOpType.mult,
                    op1=mybir.AluOpType.add,
                )
                if it < kt:
                    nc.sync.dma_start(out=slice_in, in_=xnext[:])
                    nc.gpsimd.collective_compute(
                        "AllGather",
                        mybir.AluOpType.bypass,
                        replica_groups=groups,
                        ins=[slice_in],
                        outs=[xallE],
                    )
                    nc.sync.dma_start(
                        out=xsb[:].rearrange(
                            "p (k j) f -> p k (j f)", k=NCORES),
                        in_=xallE.rearrange("(k p) f -> p k f", p=128),
                    )

            nc.sync.dma_start(out=out_d, in_=result[:])

    nc.compile()
    return nc


# ------------------------------------------------------------------ driver

_CACHE = {}


def _get_program(cfg: Cfg, S_sp: int):
    key = (cfg, S_sp)
    if key not in _CACHE:
        _CACHE[key] = _build_program(cfg, S_sp)
    return _CACHE[key]


def _prep_x(h):
    """h [N, D] -> x0 node-major [N, D] (head-interleaved feats)."""
    nheads = H
    return np.ascontiguousarray(
        h.reshape(nheads, N, d).transpose(1, 0, 2).reshape(N, D))


def _to_exchange(x0):
    """node-major [8192, 64] -> exchange layout [1024, 512]."""
    return np.ascontiguousarray(
        x0.reshape(NCORES, JBS, 128, 64).transpose(0, 2, 1, 3)
        .reshape(1024, 512))


def run(h, e, src, dst, cfg: Cfg = Cfg(), trace: bool = False):
    h = np.asarray(h, dtype=np.float32)
    e = np.asarray(e, dtype=np.float32)
    src = np.asarray(src)
    dst = np.asarray(dst)
    assert h.shape == (cfg.n, D) and e.shape == (H, E)

    tables, S_sp = _make_tables(e, src, dst, cfg)
    x0 = _prep_x(h)
    xe0 = _to_exchange(x0).astype(np.float16)
    in_maps = []
    for k in range(cfg.n_cores):
        x0s = np.ascontiguousarray(
            x0[k * 1024:(k + 1) * 1024]
            .reshape(JBS, 128, 64).transpose(1, 0, 2).reshape(128, 512))
        t = tables[k]
        in_maps.append(
            {
                "xe0": xe0,
                "x0s": x0s,
                "gmat": t["gmat"],
                "sca": t["sca"],
                "w4m": t["w4m"],
                "w4sp": t["w4sp"],
                "idx": t["idx"],
            }
        )
    nc = _get_program(cfg, S_sp)
    res = run_bass_kernel_spmd(
        nc, in_maps, list(range(cfg.n_cores)), trace=trace)
    out = np.stack([res.results[k]["out"] for k in range(cfg.n_cores)])
    # [k, p, (jb f)] -> node-major [N, 64]
    out = (out.reshape(NCORES, 128, JBS, 64).transpose(0, 2, 1, 3)
           .reshape(N, 64))
    # back to reference layout
    out = np.ascontiguousarray(
        out.reshape(N, H, d).transpose(1, 0, 2)).reshape(N, D)
    return out, res


def kernel(h, e, src, dst):
    out, _ = run(h, e, src, dst)
    return out


# revision 10
# speedup vs baseline: 3.6868x; 1.0341x over previous
"""Trainium2 Bass kernel for nn_LinearDiffusion (truncated Taylor expm(a) @ x).

Math: a = row-normalized symmetric scatter of per-head edge weights onto an
(H, N, N) zero tensor; result = sum_{i=0..6} a^i x / i! with x = h reshaped
per-head.

Strategy (8 NeuronCores, one chip) — v2, TensorE-gather:
  * x (8192 x 64 fp16, all heads together) lives in SBUF on every core; the
    per-edge gather x[src] is computed by TensorE one-hot matmuls from the
    SBUF-resident copy instead of per-edge DMA (the v1 bottleneck: GpSimd
    SWDGE descriptor generation at ~8 ns/edge, 1.7 ms total).
  * Core k owns dst rows [k*1024, (k+1)*1024) = 8 blocks of 128.  Edges are
    binned per (dst block jb, src block sb) cell; each cell gets a fixed
    64-slot capacity (mean occupancy ~64).  A 128-slot chunk = 2 cells.
      - gather:  per cell, one matmul  psum[c0:c0+64, chunk] +=
                 gmat[:, cell]^T @ xsb[:, sb, :]   (gmat: fp8 one-hot of
                 src_local, zero-padded; writes every PSUM byte -> no junk)
      - weights: one DVE multiply per 8-chunk PSUM bank with the per-head
                 w4 table (broadcast over the 16 feats of each head), fp16 out
      - scatter: per chunk, one matmul into the iteration's output bank
                 pout[:, jb*64:+64] += sca[:, chunk]^T @ xgw  (sca: fp8
                 one-hot of dst_local; PSUM accumulation across chunks)
  * Cell overflow (~5% of edges) spills to the old dma_gather path (runs on
    the otherwise-idle GpSimd, overlapped with TensorE work), gathering
    256 B token pairs from the fp16 exchange buffer; a zeroed half in the
    w4 spill table selects the wanted node of each pair, two scatter
    matmuls per spill chunk (one per parity half) accumulate into pout.
  * Between iterations: AllGather of the fp16 x slices (128 KB/rank) and a
    single strided DMA reload of xsb.  Output accumulates in fp32.
  * One SPMD program serves all 8 cores: the (jb, sb) cell grid is shared;
    only table data differs per core.
"""

import math
import os
from dataclasses import dataclass

import numpy as np

# Small (128 KB/rank) AllGathers hit the RDH algorithm cliff (~36 us each);
# mesh is ~5x faster at this size.  Must be set before NRT loads.
os.environ.setdefault("NEURON_RT_DBG_RDH_CC", "0")

import concourse.bass as bass  # noqa: F401  (kept for callers)
import concourse.tile as tile
from concourse import bacc, mybir
from concourse.bass_utils import run_bass_kernel_spmd

# ----------------------------------------------------------------- config

N, H, E, D = 8192, 4, 131072, 64
d = D // H
NCORES = 8
BLK = 128          # dst-block size == PSUM partition width
NSB = N // BLK     # src blocks (64)
CELL = 64          # slots per (jb, sb) cell
JBS = N // NCORES // BLK   # dst blocks per core (8)
MAIN_CH = NSB * CELL // 128  # main chunks per dst block (32)
K_TAYLOR = 6


@dataclass(frozen=True)
class Cfg:
    n: int = N
    n_cores: int = NCORES
    kt: int = K_TAYLOR  # Taylor depth (debug)


# ----------------------------------------------------------- preprocessing


def _entries(e, src, dst, n):
    """Unique symmetric entries with 'last write wins' duplicate semantics,
    matching jax's .at[].set() on CPU. Returns (rows, cols, w[H, nnz])."""
    src = src.astype(np.int64)
    dst = dst.astype(np.int64)
    n_edges = len(src)
    keys = np.concatenate([src * n + dst, dst * n + src])
    eid = np.concatenate([np.arange(n_edges), np.arange(n_edges)])
    order = np.arange(2 * n_edges)
    perm = np.lexsort((-order, keys))
    k_sorted = keys[perm]
    first = np.ones(len(k_sorted), dtype=bool)
    first[1:] = k_sorted[1:] != k_sorted[:-1]
    win = perm[first]
    ukeys = k_sorted[first]
    rows = (ukeys // n).astype(np.int64)
    cols = (ukeys % n).astype(np.int64)
    weids = eid[win]
    vals = e[:, weids].astype(np.float64)  # (H, nnz)
    nheads = e.shape[0]
    rowsum = np.zeros((nheads, n), dtype=np.float64)
    for hh in range(nheads):
        rowsum[hh] = np.bincount(rows, weights=vals[hh], minlength=n)
    w = (vals / rowsum[:, rows]).astype(np.float32)
    return rows, cols, w


def _make_tables(e, src, dst, cfg: Cfg):
    """Per-core device tables. Returns (tables, S_sp)."""
    import ml_dtypes

    n = cfg.n
    rows, cols, w = _entries(e, src, dst, n)
    rpc = n // cfg.n_cores

    # Pass 1: spill sizes -> shared spill chunk count S_sp.
    spill_max = 0
    percore = []
    for k in range(cfg.n_cores):
        m = (rows >= k * rpc) & (rows < (k + 1) * rpc)
        r = rows[m] - k * rpc
        c = cols[m]
        wv = w[:, m]
        jb = r // BLK
        dl = r % BLK
        sb = c // BLK
        sl = c % BLK
        # stable order by (jb, sb)
        o = np.lexsort((sb, jb))
        jb, dl, sb, sl, wv = jb[o], dl[o], sb[o], sl[o], wv[:, o]
        # rank within cell
        cell = jb * NSB + sb
        # positions within each cell (cells are contiguous after the sort)
        idx_in_cell = np.arange(len(cell)) - np.searchsorted(cell, cell)
        main = idx_in_cell < CELL
        percore.append((jb, dl, sb, sl, wv, idx_in_cell, main))
        for j in range(JBS):
            spill_max = max(spill_max, int(np.sum(~main & (jb == j))))
    S_sp = max(1, -(-spill_max // 128))  # spill chunks per dst block
    nch = MAIN_CH + S_sp                 # chunks per dst block
    ntok = JBS * S_sp * 128              # spill tokens per core

    tables = []
    for k in range(cfg.n_cores):
        jb, dl, sb, sl, wv, ic, main = percore[k]
        gmat = np.zeros((128, JBS * NSB * CELL), dtype=ml_dtypes.float8_e4m3fn)
        sca = np.zeros((128, JBS * nch * 128), dtype=ml_dtypes.float8_e4m3fn)
        w4m = np.zeros((128, JBS * MAIN_CH, H), dtype=np.float32)
        w4sp = np.zeros((128, JBS * S_sp, 2, H), dtype=np.float32)
        tok = np.zeros(ntok, dtype=np.int64)

        # main edges
        jm, dm, sbm, slm, wm, im = (
            jb[main], dl[main], sb[main], sl[main], wv[:, main], ic[main])
        # slot position within the chunk: cell parity picks the 64-half
        p = (sbm % 2) * CELL + im
        chunk_g = jm * nch + sbm // 2          # global chunk id
        gcol = jm * (NSB * CELL) + sbm * CELL + im
        gmat[slm, gcol] = 1.0
        sca[p, chunk_g * 128 + dm] = 1.0
        w4m[p, jm * MAIN_CH + sbm // 2, :] = wm.T

        # spill edges
        js, ds, ws = jb[~main], dl[~main], wv[:, ~main]
        srcs = sb[~main] * BLK + sl[~main]     # global src node
        for j in range(JBS):
            mj = js == j
            cnt = int(mj.sum())
            pos = np.arange(cnt)
            sc = pos // 128                    # spill chunk within jb
            pp = pos % 128
            src_j = srcs[mj]
            kk = src_j // rpc
            within = src_j % rpc
            jbs_ = within // BLK
            psrc = within % BLK
            token = (kk * 128 + psrc) * (JBS // 2) + jbs_ // 2
            par = jbs_ % 2
            tok[(j * S_sp + sc) * 128 + pp] = token
            w4sp[pp, j * S_sp + sc, par, :] = ws[:, mj].T
            sca[pp, (j * nch + MAIN_CH + sc) * 128 + ds[mj]] = 1.0

        # dma_gather index layout: wrapped in 16 partitions, replicated x8
        wrapped = tok.reshape(-1, 16).T.astype(np.int16)
        idx_t = np.tile(wrapped, (8, 1))
        tables.append(
            {
                "gmat": np.ascontiguousarray(gmat),
                "sca": np.ascontiguousarray(sca),
                "w4m": np.ascontiguousarray(w4m.reshape(128, -1)),
                "w4sp": np.ascontiguousarray(w4sp.reshape(128, -1)),
                "idx": np.ascontiguousarray(idx_t),
            }
        )
    return tables, S_sp


def _emulate(tables, S_sp, xe, cfg: Cfg):
    """Numpy emulation of one iteration y = A @ x using the device tables.
    xe: [1024, 512] fp16 exchange-layout x.  Returns y [8192, 64] float32."""
    nch = MAIN_CH + S_sp
    xflat = np.ascontiguousarray(xe).reshape(-1, 128)  # token rows
    out = np.zeros((NCORES, 128, JBS, 64), dtype=np.float32)
    # xsb[p, b, f] = x[b*128+p, f] ; from xe rows (k*128+p, jb*64+f)
    xsb = (
        xe.reshape(NCORES, 128, JBS, 64).transpose(1, 0, 2, 3)
        .reshape(128, NSB, 64).astype(np.float32)
    )
    for k in range(cfg.n_cores):
        t = tables[k]
        gmat = t["gmat"].astype(np.float32)
        sca = t["sca"].astype(np.float32)
        w4m = t["w4m"].reshape(128, JBS * MAIN_CH, H)
        w4sp = t["w4sp"].reshape(128, JBS * S_sp, 2, H)
        # spill gather
        seq = t["idx"][:16].T.reshape(-1)  # un-wrap
        xg_sp = xflat[seq].reshape(JBS * S_sp, 128, 128).transpose(1, 0, 2)
        for j in range(JBS):
            acc = np.zeros((128, 64), dtype=np.float32)
            for c in range(MAIN_CH):
                pg = np.zeros((128, 64), dtype=np.float32)
                for half in range(2):
                    sb = c * 2 + half
                    g = gmat[:, j * NSB * CELL + sb * CELL:][:, :CELL]
                    pg[half * 64:(half + 1) * 64] = g.T @ xsb[:, sb, :]
                w4 = w4m[:, j * MAIN_CH + c, :]  # [128, H]
                xgw = (pg.reshape(128, H, d)
                       * w4[:, :, None]).reshape(128, 64).astype(np.float16)
                s = sca[:, (j * nch + c) * 128:][:, :128]
                acc += s.T @ xgw.astype(np.float32)
            for sc in range(S_sp):
                xg = xg_sp[:, j * S_sp + sc, :].astype(np.float32)
                w4 = w4sp[:, j * S_sp + sc, :, :]  # [128, 2, H]
                xgw = (xg.reshape(128, 2, H, d)
                       * w4[:, :, :, None]).reshape(128, 128)
                xgw = xgw.astype(np.float16).astype(np.float32)
                s = sca[:, (j * nch + MAIN_CH + sc) * 128:][:, :128]
                acc += s.T @ (xgw[:, :64] + xgw[:, 64:])
            out[k, :, j, :] = acc
    # out[k, p, jb, f] -> node (k*1024 + jb*128 + p)
    return out.transpose(0, 2, 1, 3).reshape(N, 64)


# ------------------------------------------------------------ bass program

_FP32 = mybir.dt.float32
_FP16 = mybir.dt.float16
_FP8 = mybir.dt.float8e4
_I16 = mybir.dt.int16


def _build_program(cfg: Cfg, S_sp: int):
    kt = cfg.kt
    nch = MAIN_CH + S_sp
    ntok = JBS * S_sp * 128
    nc = bacc.Bacc(
        "TRN2",
        target_bir_lowering=False,
        debug=False,
        num_devices=cfg.n_cores,
    )

    xe0_d = nc.dram_tensor("xe0", [1024, 512], _FP16, kind="ExternalInput").ap()
    x0s_d = nc.dram_tensor("x0s", [128, 512], _FP32, kind="ExternalInput").ap()
    gmat_d = nc.dram_tensor(
        "gmat", [128, JBS * NSB * CELL], _FP8, kind="ExternalInput").ap()
    sca_d = nc.dram_tensor(
        "sca", [128, JBS * nch * 128], _FP8, kind="ExternalInput").ap()
    w4m_d = nc.dram_tensor(
        "w4m", [128, JBS * MAIN_CH * H], _FP32, kind="ExternalInput").ap()
    w4sp_d = nc.dram_tensor(
        "w4sp", [128, JBS * S_sp * 2 * H], _FP32, kind="ExternalInput").ap()
    idx_d = nc.dram_tensor(
        "idx", [128, ntok // 16], _I16, kind="ExternalInput").ap()
    out_d = nc.dram_tensor("out", [128, 512], _FP32, kind="ExternalOutput").ap()

    slice_in = nc.dram_tensor("slice_in", [128, 512], _FP16).ap()
    xallE = nc.dram_tensor(
        "xallE", [1024, 512], _FP16, addr_space="Shared").ap()

    groups = [list(range(cfg.n_cores))]

    with tile.TileContext(nc) as tc:
        with (
            tc.tile_pool(name="tables", bufs=1) as tp,
            tc.tile_pool(name="xgw", bufs=3) as xgwp,
            tc.tile_pool(name="psg", bufs=3, space="PSUM") as pgp,
            tc.tile_pool(name="pso", bufs=2, space="PSUM") as pop,
        ):
            gmat_sb = [tp.tile([128, NSB * CELL], _FP8, tag=f"gm{j}",
                               name=f"gmat_sb{j}")
                       for j in range(JBS)]
            sca_sb = [tp.tile([128, nch * 128], _FP8, tag=f"sc{j}",
                              name=f"sca_sb{j}")
                      for j in range(JBS)]
            w4m_sb = tp.tile([128, JBS * MAIN_CH, H], _FP32)
            w4sp_sb = tp.tile([128, JBS * S_sp, 2 * H], _FP32)
            idx_sb = tp.tile([128, ntok // 16], _I16)
            xsb = tp.tile([128, NSB, 64], _FP16)
            xnext = tp.tile([128, JBS * 64], _FP16)
            result = tp.tile([128, JBS * 64], _FP32)
            xg_sp = tp.tile([128, JBS * S_sp, 128], _FP16)
            xgw_sp = tp.tile([128, JBS * S_sp, 128], _FP16)
            z128 = tp.tile([128, 128], _FP8)
            nc.vector.memset(z128[:], 0.0)

            # small tables + x first so jb0 compute starts early
            nc.sync.dma_start(
                out=w4m_sb[:].rearrange("p c h -> p (c h)"), in_=w4m_d)
            nc.sync.dma_start(
                out=w4sp_sb[:].rearrange("p c h -> p (c h)"), in_=w4sp_d)
            nc.sync.dma_start(out=idx_sb[:], in_=idx_d)
            # x0 into xsb ([p, b, f] <- xe0 rows (k*128+p, jb*64+f))
            nc.sync.dma_start(
                out=xsb[:].rearrange("p (k j) f -> p k (j f)", k=NCORES),
                in_=xe0_d.rearrange("(k p) f -> p k f", p=128),
            )
            nc.sync.dma_start(out=result[:], in_=x0s_d)
            for j in range(JBS):
                nc.sync.dma_start(
                    out=gmat_sb[j][:],
                    in_=gmat_d[:, j * NSB * CELL:(j + 1) * NSB * CELL])
                nc.sync.dma_start(
                    out=sca_sb[j][:],
                    in_=sca_d[:, j * nch * 128:(j + 1) * nch * 128])

            for it in range(1, kt + 1):
                coef = 1.0 / math.factorial(it)
                xsrc = xe0_d if it == 1 else xallE
                # ---- spill gather (GpSimd) + weight multiply
                nc.gpsimd.dma_gather(
                    xg_sp[:],
                    xsrc.rearrange("r (q e) -> (r q) e", e=128),
                    idx_sb[:],
                    ntok,
                    ntok,
                    128,
                    single_packet=False,
                )
                for par in range(2):
                    xg4 = xg_sp[:, :, par * 64:(par + 1) * 64].rearrange(
                        "p c (h f) -> p c h f", h=H)
                    wv = (
                        w4sp_sb[:]
                        .rearrange("p c (t h) -> p c t h", t=2)[:, :, par, :]
                        .unsqueeze(3)
                        .to_broadcast([128, JBS * S_sp, H, d])
                    )
                    og = xgw_sp[:, :, par * 64:(par + 1) * 64].rearrange(
                        "p c (h f) -> p c h f", h=H)
                    nc.vector.tensor_mul(og, xg4, wv)

                pout = pop.tile([128, JBS * 64], _FP32, tag="pout")
                # open the accumulation bank: zero matmul writes every byte
                # (hw clears has_written per-element, not per-bank)
                nc.tensor.matmul(
                    pout[:],
                    lhsT=z128[:],
                    rhs=xsb[:, 0:8, :],
                    start=True,
                    stop=False,
                )
                # ---- main cells
                for j in range(JBS):
                    for g in range(MAIN_CH // 8):
                        pg = pgp.tile([128, 8 * 64], _FP32, tag="pg")
                        for ci in range(8):
                            c = g * 8 + ci
                            for half in range(2):
                                sb = c * 2 + half
                                nc.tensor.matmul(
                                    pg[half * 64:(half + 1) * 64,
                                       ci * 64:(ci + 1) * 64],
                                    lhsT=gmat_sb[j][
                                        :, sb * CELL:(sb + 1) * CELL],
                                    rhs=xsb[:, sb, :],
                                    start=True,
                                    stop=True,
                                )
                        xgw = xgwp.tile([128, 8, 64], _FP16, tag="xgw")
                        pg4 = pg[:].rearrange("p (c h f) -> p c h f", c=8, h=H)
                        wv = (
                            w4m_sb[:, j * MAIN_CH + g * 8:
                                   j * MAIN_CH + (g + 1) * 8, :]
                            .unsqueeze(3)
                            .to_broadcast([128, 8, H, d])
                        )
                        nc.vector.tensor_mul(
                            xgw[:].rearrange("p c (h f) -> p c h f", h=H),
                            pg4, wv)
                        for ci in range(8):
                            c = g * 8 + ci
                            nc.tensor.matmul(
                                pout[:, j * 64:(j + 1) * 64],
                                lhsT=sca_sb[j][:, c * 128:(c + 1) * 128],
                                rhs=xgw[:, ci, :],
                                start=False,
                                stop=False,
                            )
                # ---- spill scatter (end of iteration; rhs ready early)
                for j in range(JBS):
                    for sc in range(S_sp):
                        for par in range(2):
                            last = (j == JBS - 1 and sc == S_sp - 1
                                    and par == 1)
                            nc.tensor.matmul(
                                pout[:, j * 64:(j + 1) * 64],
                                lhsT=sca_sb[j][
                                    :, (MAIN_CH + sc) * 128:
                                    (MAIN_CH + sc + 1) * 128],
                                rhs=xgw_sp[:, j * S_sp + sc,
                                           par * 64:(par + 1) * 64],
                                start=False,
                                stop=last,
                            )
                # ---- evacuate + Taylor accumulate
                nc.scalar.copy(xnext[:], pout[:])
                nc.vector.scalar_tensor_tensor(
                    result[:],
                    pout[:],
                    coef,
                    result[:],
                    op0=mybir.AluOpType.mult,
                    op1=mybir.AluOpType.add,
                )
                if it < kt:
                    nc.sync.dma_start(out=slice_in, in_=xnext[:])
                    nc.gpsimd.collective_compute(
                        "AllGather",
                        mybir.AluOpType.bypass,
                        replica_groups=groups,
                        ins=[slice_in],
                        outs=[xallE],
                    )
                    nc.sync.dma_start(
                        out=xsb[:].rearrange(
                            "p (k j) f -> p k (j f)", k=NCORES),
                        in_=xallE.rearrange("(k p) f -> p k f", p=128),
                    )

            nc.sync.dma_start(out=out_d, in_=result[:])

    nc.compile()
    return nc


# ------------------------------------------------------------------ driver

_CACHE = {}


def _get_program(cfg: Cfg, S_sp: int):
    key = (cfg, S_sp)
    if key not in _CACHE:
        _CACHE[key] = _build_program(cfg, S_sp)
    return _CACHE[key]


def _prep_x(h):
    """h [N, D] -> x0 node-major [N, D] (head-interleaved feats)."""
    nheads = H
    return np.ascontiguousarray(
        h.reshape(nheads, N, d).transpose(1, 0, 2).reshape(N, D))


def _to_exchange(x0):
    """node-major [8192, 64] -> exchange layout [1024, 512]."""
    return np.ascontiguousarray(
        x0.reshape(NCORES, JBS, 128, 64).transpose(0, 2, 1, 3)
        .reshape(1024, 512))


def run(h, e, src, dst, cfg: Cfg = Cfg(), trace: bool = False):
    h = np.asarray(h, dtype=np.float32)
    e = np.asarray(e, dtype=np.float32)
    src = np.asarray(src)
    dst = np.asarray(dst)
    assert h.shape == (cfg.n, D) and e.shape == (H, E)

    tables, S_sp = _make_tables(e, src, dst, cfg)
    x0 = _prep_x(h)
    xe0 = _to_exchange(x0).astype(np.float16)
    in_maps = []
    for k in range(cfg.n_cores):
        x0s = np.ascontiguousarray(
            x0[k * 1024:(k + 1) * 1024]
            .reshape(JBS, 128, 64).transpose(1, 0, 2).reshape(128, 512))
        t = tables[k]
        in_maps.append(
            {
                "xe0": xe0,
                "x0s": x0s,
                "gmat": t["gmat"],
                "sca": t["sca"],
                "w4m": t["w4m"],
                "w4sp": t["w4sp"],
                "idx": t["idx"],
            }
        )
    nc = _get_program(cfg, S_sp)
    res = run_bass_kernel_spmd(
        nc, in_maps, list(range(cfg.n_cores)), trace=trace)
    out = np.stack([res.results[k]["out"] for k in range(cfg.n_cores)])
    # [k, p, (jb f)] -> node-major [N, 64]
    out = (out.reshape(NCORES, 128, JBS, 64).transpose(0, 2, 1, 3)
           .reshape(N, 64))
    # back to reference layout
    out = np.ascontiguousarray(
        out.reshape(N, H, d).transpose(1, 0, 2)).reshape(N, D)
    return out, res


def kernel(h, e, src, dst):
    out, _ = run(h, e, src, dst)
    return out
